# revision 6
# baseline (speedup 1.0000x reference)
"""EuclideanVisitEncoder forward: masked-mean embedding bag on 8 NeuronCores.

out[b, :] = sum_l (ids[b,l] != 0) * T[ids[b,l], :] / max(count_b, 1)

Sharding: data-parallel over the batch across 8 cores (25088 padded rows
each); the 6.4 MB table is replicated (stays in each core's DRAM; row 0 is
zeroed host-side so pad ids gather zeros).

Per core the kernel loops (hardware For_i) over 196 tiles of 128 rows.
Each tile: DMA the [128, 64] id block, issue 64 vector-indirect DMA
gathers (one per sequence position; each gathers 128 table rows - one per
partition - into a column slice of a [128, 64*16] SBUF tile), then a DVE
reduction over l, a valid-count, reciprocal, scale, and the output store.

The indirect gather (qPoolDynamic, one offset per partition) is the only
data-dependent addressing primitive available on this image (loadable Q7
ucode libraries - dma_gather etc. - are excluded), and its measured cost
~1.3 us per 128 gathered rows dominates the runtime.
"""

import numpy as np

PAD_IDX = 0
NUM_CODES = 100000
DIM = 16
B, L = 200000, 64
N_CORES = 8

TILE_ROWS = 128
B_SHARD = 25088                       # 25000 + pad to multiple of 128
N_TILES = B_SHARD // TILE_ROWS        # 196

_PROGRAM_CACHE = {}


def build_program(b_shard=B_SHARD, repeats=1, unroll=28, bufs=2):
    from contextlib import ExitStack

    import concourse.tile as tile
    from concourse import bacc, bass, mybir

    n_tiles = b_shard // TILE_ROWS
    assert b_shard % TILE_ROWS == 0 and n_tiles % unroll == 0

    nc = bacc.Bacc("TRN2", target_bir_lowering=False, debug=False)
    ids_t = nc.dram_tensor("code_ids", [b_shard, L], mybir.dt.int32, kind="ExternalInput")
    tbl_t = nc.dram_tensor("emb_weight", [NUM_CODES, DIM], mybir.dt.float32, kind="ExternalInput")
    out_t = nc.dram_tensor("out", [b_shard, DIM], mybir.dt.float32, kind="ExternalOutput")

    with ExitStack() as ctx:
        tc = ctx.enter_context(tile.TileContext(nc))
        ids_pool = ctx.enter_context(tc.tile_pool(name="ids", bufs=bufs))
        g_pool = ctx.enter_context(tc.tile_pool(name="g", bufs=bufs))
        s_pool = ctx.enter_context(tc.tile_pool(name="s", bufs=bufs))

        def tile_body(row0_expr):
            """row0_expr: element row offset (ScalarValue expr or int)."""
            ids_tile = ids_pool.tile([128, L], mybir.dt.int32, tag="ids", name=f"ids{nc.next_id()}")
            nc.sync.dma_start(out=ids_tile[:], in_=ids_t[bass.ds(row0_expr, 128), :])
            g = g_pool.tile([128, L * DIM], mybir.dt.float32, tag="g", name=f"g{nc.next_id()}")
            for l in range(L):
                nc.gpsimd.indirect_dma_start(
                    out=g[:, l * DIM : (l + 1) * DIM],
                    out_offset=None,
                    in_=tbl_t[:, :],
                    in_offset=bass.IndirectOffsetOnAxis(ap=ids_tile[:, l : l + 1], axis=0),
                )
            mask = s_pool.tile([128, L], mybir.dt.float32, tag="mask", name=f"m{nc.next_id()}")
            nc.vector.tensor_scalar(mask[:], ids_tile[:], 0, None, op0=mybir.AluOpType.not_equal)
            den = s_pool.tile([128, 1], mybir.dt.float32, tag="den", name=f"d{nc.next_id()}")
            nc.vector.tensor_reduce(den[:], mask[:], axis=mybir.AxisListType.X, op=mybir.AluOpType.add)
            nc.vector.tensor_scalar_max(den[:], den[:], 1.0)
            recip = s_pool.tile([128, 1], mybir.dt.float32, tag="recip", name=f"r{nc.next_id()}")
            nc.vector.reciprocal(recip[:], den[:])
            acc = s_pool.tile([128, DIM], mybir.dt.float32, tag="acc", name=f"a{nc.next_id()}")
            nc.vector.tensor_reduce(
                acc[:],
                g[:].rearrange("p (l d) -> p d l", l=L, d=DIM),
                axis=mybir.AxisListType.X,
                op=mybir.AluOpType.add,
            )
            outt = s_pool.tile([128, DIM], mybir.dt.float32, tag="outt", name=f"o{nc.next_id()}")
            nc.vector.tensor_scalar(outt[:], acc[:], recip[:], None, op0=mybir.AluOpType.mult)
            nc.sync.dma_start(out=out_t[bass.ds(row0_expr, 128), :], in_=outt[:])

        if repeats == 1:
            with tc.For_i(0, n_tiles // unroll, 1) as i:
                for u in range(unroll):
                    tile_body(i * (128 * unroll) + u * 128)
        else:
            # timing variant: repeat the whole shard computation
            with tc.For_i(0, repeats, 1) as _r:
                with tc.For_i(0, n_tiles // unroll, 1) as i:
                    for u in range(unroll):
                        tile_body(i * (128 * unroll) + u * 128)

    nc.compile()
    return nc


def _get_program():
    key = (B_SHARD, 1)
    if key not in _PROGRAM_CACHE:
        _PROGRAM_CACHE[key] = build_program()
    return _PROGRAM_CACHE[key]


def make_in_maps(code_ids: np.ndarray, emb_weight: np.ndarray):
    code_ids = np.ascontiguousarray(np.asarray(code_ids), dtype=np.int32)
    emb_weight = np.ascontiguousarray(np.asarray(emb_weight), dtype=np.float32)
    tbl = emb_weight.copy()
    tbl[PAD_IDX, :] = 0.0
    b_total = N_CORES * B_SHARD
    ids_pad = np.zeros((b_total, L), dtype=np.int32)
    ids_pad[: code_ids.shape[0], :] = code_ids
    return [
        {
            "code_ids": ids_pad[i * B_SHARD : (i + 1) * B_SHARD],
            "emb_weight": tbl,
        }
        for i in range(N_CORES)
    ]


def kernel(code_ids: np.ndarray, emb_weight: np.ndarray, **kwargs) -> np.ndarray:
    from concourse import bass_utils

    nc = _get_program()
    in_maps = make_in_maps(code_ids, emb_weight)
    res = bass_utils.run_bass_kernel_spmd(nc, in_maps, core_ids=list(range(N_CORES)))
    out = np.concatenate([res.results[i]["out"] for i in range(N_CORES)], axis=0)
    return out[: np.asarray(code_ids).shape[0]]


if __name__ == "__main__":
    rng = np.random.default_rng(0)
    ids = rng.integers(0, NUM_CODES, size=(B, L)).astype(np.int32)
    w = rng.standard_normal((NUM_CODES, DIM)).astype(np.float32)
    o = kernel(code_ids=ids, emb_weight=w)
    print(o.shape, o.dtype, o[:2, :4])
